# revision 17
# baseline (speedup 1.0000x reference)
"""BotRGCN on 8 trn2 NeuronCores (SPMD, raw Bacc).

Nodes row-sharded 8 ways (12500/core, padded to 12800). Phase A
(768->8 projections etc.) fully sharded with PE transposes + bf16
matmuls. RGCN layers: AllGather bf16 node features -> shared gather
table; per-relation degree-sorted ELL plane gathers via indirect DMA
(int32 idx, 64B rows); DVE accumulate; per-node 1/cnt scale; DRAM
unpermute; dense matmuls in feature-major (^T) space.
"""
import sys
sys.path.insert(0, "/opt/trn_rl_repo")
from contextlib import ExitStack

import numpy as np
import ml_dtypes

from concourse import bacc, bass, mybir
from concourse import library_config
from concourse.bass_utils import run_bass_kernel_spmd

F32 = mybir.dt.float32
BF16 = mybir.dt.bfloat16
I32 = mybir.dt.int32
LRELU = mybir.ActivationFunctionType.Lrelu
ACOPY = mybir.ActivationFunctionType.Copy

N_CORES = 8
NUM_REL = 2
CBUF = 96          # gather piece size (idx columns per indirect DMA)


class Cfg:
    def __init__(self, shard_real, shard_pad, pieces):
        self.shard_real = shard_real
        self.shard_pad = shard_pad
        self.n_super = shard_pad // 512
        self.nt = shard_pad // 128
        self.c_unp = shard_pad // 128
        self.tabv = N_CORES * shard_pad
        # pieces[r] = list of (idx_col0, ck, adds) ; adds = [(agg_blk0, msg_blk0, nblk)]
        self.pieces = pieces
        self.ctot = (sum(ck for p in pieces for (_, ck, _) in [p[1]] for p in [p]) if False
                     else None)


def build_bass(cfg: Cfg):
    nc = bacc.Bacc("TRN2", debug=False)
    mmctx = ExitStack()
    SP = cfg.shard_pad
    NT = cfg.nt
    NS = cfg.n_super
    TABV = cfg.tabv
    CUNP = cfg.c_unp
    n_gcols = max(c0 + ck for r in range(NUM_REL) for (c0, ck, _) in cfg.pieces[r]) \
        if any(cfg.pieces) else 0
    CTOT = n_gcols + 2 * CUNP

    desT_in = nc.declare_dram_parameter("desT", [128, 6 * SP], BF16, isOutput=False)
    twT_in = nc.declare_dram_parameter("twT", [128, 6 * SP], BF16, isOutput=False)
    nct_in = nc.declare_dram_parameter("nct", [16, SP], BF16, isOutput=False)
    idx_in = nc.declare_dram_parameter("idx", [128, CTOT], I32, isOutput=False)
    coef_in = nc.declare_dram_parameter("coefnm", [128, NUM_REL, NT], F32, isOutput=False)
    wd_in = nc.declare_dram_parameter("wd", [768, 32], BF16, isOutput=False)
    wt_in = nc.declare_dram_parameter("wt", [768, 32], BF16, isOutput=False)
    wnc_in = nc.declare_dram_parameter("wnc", [16, 32], BF16, isOutput=False)
    wsm_in = nc.declare_dram_parameter("wsm", [6, 33, 32], BF16, isOutput=False)
    id128_in = nc.declare_dram_parameter("id128", [128, 128], F32, isOutput=False)
    id32_in = nc.declare_dram_parameter("id32", [32, 32], BF16, isOutput=False)
    out_ext = nc.declare_dram_parameter("out", [2, SP], F32, isOutput=True)

    shard_ag = nc.dram_tensor("shard_ag", [SP, 32], BF16)
    table = nc.dram_tensor("table", [TABV, 32], BF16, addr_space="Shared")
    scratch = nc.dram_tensor("scratch", [SP, 32], F32)

    live = []

    def sb(name, shape, dt):
        cm = nc.sbuf_tensor(name, shape, dt)
        t = cm.__enter__()
        live.append(cm)
        return t

    def psum_dt(name, shape, dt):
        cm = nc.psum_tensor(name, shape, dt)
        t = cm.__enter__()
        live.append(cm)
        return t

    def psum(name, shape):
        return psum_dt(name, shape, F32)

    sb_dT = sb("sb_dT", [128, 2, 6, 512], BF16)
    sb_tT = sb("sb_tT", [128, 2, 6, 512], BF16)
    sb_ncT = sb("sb_ncT", [16, SP], BF16)
    xT = sb("xT", [33, SP], BF16)
    aggT0 = sb("aggT0", [32, SP], BF16)
    aggT1 = sb("aggT1", [32, SP], BF16)
    aggb = [sb("agg0", [128, NT * 32], F32), sb("agg1", [128, NT * 32], F32)]
    sb("pad_align", [128, 3584], mybir.dt.uint8)
    msgs2 = [sb("msgsA", [128, CBUF * 32], BF16), sb("msgsB", [128, CBUF * 32], BF16)]
    sb_shard = sb("sb_shard", [128, NT, 32], BF16)
    sb_idxg = [[sb(f"sb_idxg{r}_{pi}", [128, ck], I32)
                for pi, (c0, ck, _) in enumerate(cfg.pieces[r])] for r in range(NUM_REL)]
    sb_idxu = [sb(f"sb_idxu{r}", [128, CUNP], I32) for r in range(NUM_REL)]
    sb_coef = sb("sb_coef", [128, NUM_REL, NT], F32)
    sb_wd = sb("sb_wd", [128, 6, 32], BF16)
    sb_wt = sb("sb_wt", [128, 6, 32], BF16)
    sb_wnc = sb("sb_wnc", [16, 32], BF16)
    sb_wsm = sb("sb_wsm", [33, 6, 32], BF16)
    sb_id128 = sb("sb_id128", [128, 128], F32)
    sb_id32 = sb("sb_id32", [32, 32], BF16)
    sb_x3T = sb("sb_x3T", [33, 512], BF16)
    sb_lg = sb("sb_lg", [2, 2, 512], F32)

    pb = [psum(f"pb{i}", [128, 512]) for i in range(8)]
    pbx = pb[5][:, :].bitcast(BF16)

    plan = {"sync": [], "pe": [], "act": [], "dve": [], "gp": []}

    def op(engine, fn):
        plan[engine].append(fn)

    class Sem:
        def __init__(self, name):
            cm = nc.semaphore(name)
            self.h = cm.__enter__()
            live.append(cm)
            self.n = 0

        def inc(self, inst, k=1):
            # runtime half: attach the semaphore update (no counting here)
            inst.then_inc(self.h, k)

        def pinc(self, k=1):
            # plan-time half: advance the cumulative count
            self.n += k
            return self.n

    s_load = Sem("s_load")
    s_a1 = Sem("s_a1")
    s_ld = [Sem("s_ld0"), Sem("s_ld1")]
    s_lr = Sem("s_lr")
    s_gq = [Sem(f"s_gq{i}") for i in range(8)]
    s_tp = Sem("s_tp")
    s_cp = Sem("s_cp")
    s_mm = Sem("s_mm")
    s_x1 = Sem("s_x1")
    s_gp = Sem("s_gp")
    s_cc = Sem("s_cc")
    s_dve = Sem("s_dve")
    s_sh = Sem("s_sh")

    def W(engine, sem, val):
        if val > 0:
            op(engine, lambda eng, s=sem, v=val: eng.wait_ge(s.h, v))

    # ---------------- constants ----------------
    def c_loads(eng):
        for r in range(NUM_REL):
            for pi, (c0, ck, _) in enumerate(cfg.pieces[r]):
                eng.dma_start(out=sb_idxg[r][pi][:], in_=idx_in[:, c0:c0 + ck]).then_inc(s_load.h, 16)
            u0 = n_gcols + r * CUNP
            eng.dma_start(out=sb_idxu[r][:], in_=idx_in[:, u0:u0 + CUNP]).then_inc(s_load.h, 16)
        eng.dma_start(out=sb_coef[:], in_=coef_in[:, :, :]).then_inc(s_load.h, 16)
        eng.dma_start(out=sb_wd[:], in_=wd_in.ap().rearrange("(c p) m -> p c m", p=128)).then_inc(s_load.h, 16)
        eng.dma_start(out=sb_wt[:], in_=wt_in.ap().rearrange("(c p) m -> p c m", p=128)).then_inc(s_load.h, 16)
        eng.dma_start(out=sb_wnc[:], in_=wnc_in[:, :]).then_inc(s_load.h, 16)
        eng.dma_start(out=sb_wsm[:], in_=wsm_in.ap().rearrange("c p m -> p c m")).then_inc(s_load.h, 16)
        eng.dma_start(out=sb_ncT[:], in_=nct_in[:, :]).then_inc(s_load.h, 16)
        eng.dma_start(out=sb_id128[:], in_=id128_in[:, :]).then_inc(s_load.h, 16)
        eng.dma_start(out=sb_id32[:], in_=id32_in[:, :]).then_inc(s_load.h, 16)
    op("sync", c_loads)
    s_load.n += (8 + sum(len(p) for p in cfg.pieces) + NUM_REL) * 16
    NCONST = s_load.n

    op("gp", lambda eng: eng.load_library(library_config.mlp))

    def init_ones(eng):
        eng.memset(xT[32:33, :], 1.0)
        s_dve.inc(eng.memset(sb_x3T[32:33, :], 1.0))
    op("dve", init_ones)
    s_dve.pinc()
    NINIT = s_dve.n

    # =======================================================
    # Phase A: xT = lrelu(lrelu([d|t|n|c]) @ Wi + bi), inputs pre-transposed bf16
    # =======================================================
    # Software-pipelined: step v runs mm(v) | wi(v-1) | x1t(v-2) on PE and
    # a1(v) | a2(v-1) | act_sh(v-2) on ACT. pb6 rotates over 3 banks, pb7 over 2.
    mm_no = {}
    wi_no = {}
    for v in range(NS + 2):
        if v < NS:
            buf = v % 2

            def ld(eng, v=v, buf=buf):
                eng.dma_start(
                    out=sb_dT[:, buf, :, :],
                    in_=desT_in.ap().rearrange("p (c n) -> p c n", n=SP)[:, :, v * 512:(v + 1) * 512],
                ).then_inc(s_ld[buf].h, 16)
                eng.dma_start(
                    out=sb_tT[:, buf, :, :],
                    in_=twT_in.ap().rearrange("p (c n) -> p c n", n=SP)[:, :, v * 512:(v + 1) * 512],
                ).then_inc(s_ld[buf].h, 16)
            if v == 2:
                W("sync", s_mm, 1)
            elif v >= 3:
                W("sync", s_mm, 2 * v - 4)
            op("sync", ld)
            s_ld[buf].n += 32

        # ---- PE ----
        if v < NS:
            if v == 0:
                W("pe", s_load, NCONST)
                W("pe", s_dve, NINIT)
            W("pe", s_ld[v % 2], 32 * (v // 2 + 1))
            if v >= 3:
                W("pe", s_a1, v - 2)       # a1(v-3) freed this pb6 bank
            b6 = pb[v % 3]

            def pe_mm(eng, v=v, b6=b6):
                for c in range(6):
                    nc.tensor.matmul(b6[0:32, :], sb_wd[:, c, :], sb_dT[:, v % 2, c, :],
                                     start=(c == 0), stop=False)
                for c in range(6):
                    nc.tensor.matmul(b6[0:32, :], sb_wt[:, c, :], sb_tT[:, v % 2, c, :],
                                     start=False, stop=False)
                last = nc.tensor.matmul(b6[0:32, :], sb_wnc[:, :],
                                        sb_ncT[:, v * 512:(v + 1) * 512],
                                        start=False, stop=True)
                s_mm.inc(last)
            op("pe", pe_mm)
            mm_no[v] = s_mm.pinc()

        if v >= 1 and v - 1 < NS:
            W("pe", s_a1, v)               # a1(v-1) produced xT slice
            if v >= 3:
                W("pe", s_x1, v - 2)       # a2(v-3) freed this pb7 bank
            b7 = pb[3 + (v - 1) % 2]

            def pe_wi(eng, v=v, b7=b7):
                s_mm.inc(nc.tensor.matmul(b7[0:32, :], sb_wsm[:, 0, :],
                                          xT[0:33, (v - 1) * 512:v * 512],
                                          start=True, stop=True))
            op("pe", pe_wi)
            wi_no[v - 1] = s_mm.pinc()

        if v >= 2 and v - 2 < NS:
            i = v - 2
            if i == NS - 1 and cfg.shard_real < SP:
                W("dve", s_x1, NS)
                op("dve", lambda eng: s_dve.inc(eng.memset(xT[0:32, cfg.shard_real:SP], 0)))
                s_dve.pinc()
                W("pe", s_dve, NINIT + 1)
            W("pe", s_x1, i + 1)           # a2(i) done
            if i >= 1:
                W("pe", s_sh, i)           # act_sh(i-1) freed pbx

            def pe_x1t(eng, i=i):
                last = None
                for t in range(4):
                    last = nc.tensor.transpose(
                        out=pbx[:, t * 32:(t + 1) * 32],
                        in_=xT[0:32, i * 512 + t * 128:i * 512 + (t + 1) * 128],
                        identity=sb_id32[:])
                s_tp.inc(last)
            op("pe", pe_x1t)
            s_tp.pinc()

        # ---- ACT ----
        if v < NS:
            W("act", s_mm, mm_no[v])

            def a1(eng, v=v, b6=pb[v % 3]):
                s_a1.inc(eng.activation(out=xT[0:32, v * 512:(v + 1) * 512],
                                        in_=b6[0:32, :], func=LRELU, alpha=0.01))
            op("act", a1)
            s_a1.pinc()

        if v >= 1 and v - 1 < NS:
            W("act", s_mm, wi_no[v - 1])

            def a2(eng, v=v, b7=pb[3 + (v - 1) % 2]):
                s_x1.inc(eng.activation(out=xT[0:32, (v - 1) * 512:v * 512],
                                        in_=b7[0:32, :], func=LRELU, alpha=0.01))
            op("act", a2)
            s_x1.pinc()

        if v >= 2 and v - 2 < NS:
            i = v - 2
            W("act", s_tp, i + 1)

            def act_sh(eng, i=i):
                s_sh.inc(eng.activation(
                    out=sb_shard[:, 4 * i:4 * i + 4, :].rearrange("p a b -> p (a b)"),
                    in_=pbx[:, 0:128], func=ACOPY))
            op("act", act_sh)
            s_sh.pinc()

    # =======================================================
    # RGCN layers
    # =======================================================
    gq_rr = [0]
    unp_done = [0]
    tp_done_r = {0: 0, 1: 0}

    def emit_layer(layer):
        # shard -> shard_ag in 5 chunks on the (idle) sync engine, each fired
        # as soon as its act_sh slices are done -- overlaps the tail compute
        S_base = s_sh.n - NS      # act_sh counts for this layer start here
        if layer == 2:
            W("sync", s_cc, s_cc.n)   # AllGather-1 consumed shard_ag
        for chk in range(5):
            t0, t1 = chk * (NT // 5), (chk + 1) * (NT // 5)
            W("sync", s_sh, S_base + (t1 // 4))

            def shard_chunk(eng, t0=t0, t1=t1):
                s_gp.inc(eng.dma_start(
                    out=shard_ag[:, :].rearrange("(t p) d -> p t d", p=128)[:, t0:t1, :],
                    in_=sb_shard[:, t0:t1, :]), 16)
            op("sync", shard_chunk)
            s_gp.pinc(16)
        gp_shard_done = s_gp.n
        W("gp", s_gp, gp_shard_done)
        op("gp", lambda eng: s_cc.inc(eng.collective_compute(
            "AllGather", mybir.AluOpType.bypass,
            ins=[shard_ag[:, :]], outs=[table[:, :]],
            replica_groups=[list(range(N_CORES))])))
        s_cc.pinc()
        W("gp", s_cc, s_cc.n)

        # hoisted memsets: each relation's agg buffer cleared as soon as its
        # previous-layer transposes have consumed it
        D0r = {}
        for r in range(NUM_REL):
            W("dve", s_tp, tp_done_r[r])
            op("dve", lambda eng, r=r: s_dve.inc(eng.memset(aggb[r][:, :], 0)))
            D0r[r] = s_dve.pinc()

        for r in range(NUM_REL):
            ag = aggb[r]
            # accs for this relation start at inc D0+1 (both memsets precede r0's)
            D0 = D0r[1] if r == 0 else s_dve.n
            for pi, (c0, ck, adds) in enumerate(cfg.pieces[r]):
                pbuf = pi % 2
                sq = (gq_rr[0]) % 8
                gq_rr[0] += 1
                # gather pi waits DVE finished consuming msgs[pbuf] (piece pi-2)
                W("gp", s_dve, D0 + pi - 1 if pi >= 2 else D0)

                def gth(eng, r=r, pi=pi, ck=ck, pbuf=pbuf, sq=sq):
                    for j in range(ck):
                        inst = eng.indirect_dma_start(
                            out=msgs2[pbuf][:, j * 32:(j + 1) * 32],
                            out_offset=None,
                            in_=table[:, :],
                            in_offset=bass.IndirectOffsetOnAxis(
                                ap=sb_idxg[r][pi][:, j:j + 1], axis=0),
                        )
                        s_gq[sq].inc(inst, 16)
                op("gp", gth)
                s_gq[sq].pinc(16 * ck)
                W("dve", s_gq[sq], s_gq[sq].n)

                def acc(eng, adds=adds, pbuf=pbuf, ag=ag):
                    last = None
                    for j, (ab, mb, nb) in enumerate(adds):
                        if j > 0:
                            eng.drain()
                        last = nc.vector.tensor_tensor(
                            out=ag[:, ab * 32:(ab + nb) * 32],
                            in0=ag[:, ab * 32:(ab + nb) * 32],
                            in1=msgs2[pbuf][:, mb * 32:(mb + nb) * 32],
                            op=mybir.AluOpType.add)
                    s_dve.inc(last)
                op("dve", acc)
                s_dve.pinc()

            # scale by 1/cnt (sorted order), per node-tile
            def scl(eng, r=r, ag=ag):
                eng.drain()
                last = None
                for t in range(NT):
                    last = nc.vector.tensor_tensor(
                        out=ag[:, t * 32:(t + 1) * 32],
                        in0=ag[:, t * 32:(t + 1) * 32],
                        in1=sb_coef[:, r, t:t + 1].to_broadcast([128, 32]),
                        op=mybir.AluOpType.mult)
                s_dve.inc(last)
            op("dve", scl)
            s_dve.pinc()

            if r == 0:
                # r0 is already canonical: transpose agg0 -> aggT0 right away
                W("pe", s_dve, s_dve.n)
                C0 = s_cp.n
                T0 = s_tp.n
                for g in range(NT // 4):
                    bank = pb[1 + (g % 2)]
                    W("pe", s_cp, C0 + g - 1 if g >= 2 else C0)

                    def pe_at(eng, g=g, bank=bank, ag=ag):
                        last = None
                        for t in range(4):
                            n = g * 4 + t
                            last = nc.tensor.transpose(out=bank[0:32, t * 128:(t + 1) * 128],
                                                       in_=ag[:, n * 32:(n + 1) * 32],
                                                       identity=sb_id128[:])
                        s_tp.inc(last)
                    op("pe", pe_at)
                    s_tp.pinc()
                    W("act", s_tp, T0 + g + 1)

                    def act_at(eng, g=g, bank=bank):
                        s_cp.inc(eng.activation(out=aggT0[:, g * 512:(g + 1) * 512],
                                                in_=bank[0:32, :], func=ACOPY))
                    op("act", act_at)
                    s_cp.pinc()
                tp_done_r[0] = s_tp.n

        # r1: scratch round trip (sorted -> sigma0), then interleaved
        # unp -> transpose -> tail, chunk by chunk
        if layer == 2:
            W("sync", s_gp, unp_done[0])   # layer-1 unp consumed scratch
        W("sync", s_dve, s_dve.n)
        op("sync", lambda eng: s_gp.inc(
            eng.dma_start(out=scratch[:, :].rearrange("(t p) d -> p t d", p=128),
                          in_=aggb[1][:, :].rearrange("p (t d) -> p t d", d=32)), 16))
        s_gp.pinc(16)
        W("gp", s_gp, s_gp.n)
        U0 = s_gp.n

        def unp(eng):
            for j in range(CUNP):
                inst = eng.indirect_dma_start(
                    out=aggb[1][:, j * 32:(j + 1) * 32],
                    out_offset=None,
                    in_=scratch[:, :],
                    in_offset=bass.IndirectOffsetOnAxis(ap=sb_idxu[1][:, j:j + 1], axis=0),
                )
                s_gp.inc(inst, 16)
        op("gp", unp)
        s_gp.pinc(16 * CUNP)
        unp_done[0] = s_gp.n

        cp_r1 = s_cp.n
        T0 = s_tp.n
        X0 = s_x1.n
        for g in range(NT // 4 + 1):
            if g < NT // 4:
                bank = pb[1 + (g % 2)]
                W("pe", s_cp, cp_r1 + g - 1 if g >= 2 else cp_r1)
                W("pe", s_gp, U0 + 16 * (4 * g + 4))

                def pe_at1(eng, g=g, bank=bank):
                    last = None
                    for t in range(4):
                        n = g * 4 + t
                        last = nc.tensor.transpose(out=bank[0:32, t * 128:(t + 1) * 128],
                                                   in_=aggb[1][:, n * 32:(n + 1) * 32],
                                                   identity=sb_id128[:])
                    s_tp.inc(last)
                op("pe", pe_at1)
                s_tp.pinc()
                W("act", s_tp, T0 + g + 1)

                def act_at1(eng, g=g, bank=bank):
                    s_cp.inc(eng.activation(out=aggT1[:, g * 512:(g + 1) * 512],
                                            in_=bank[0:32, :], func=ACOPY))
                op("act", act_at1)
                s_cp.pinc()
            if g >= 1:
                ch = g - 1
                bank2 = pb[3 + (ch % 2)]
                if ch >= 2:
                    W("pe", s_x1, X0 + ch - 1)
                else:
                    W("pe", s_x1, X0)
                W("pe", s_cp, cp_r1 + ch + 1)

                def pe_tail(eng, ch=ch, bank=bank2):
                    sl = slice(ch * 512, (ch + 1) * 512)
                    nc.tensor.matmul(bank[0:32, :], sb_wsm[:, 1, :], xT[0:33, sl],
                                     start=True, stop=False)
                    nc.tensor.matmul(bank[0:32, :], sb_wsm[0:32, 2, :], aggT0[:, sl],
                                     start=False, stop=False)
                    last = nc.tensor.matmul(bank[0:32, :], sb_wsm[0:32, 3, :], aggT1[:, sl],
                                            start=False, stop=True)
                    s_mm.inc(last)
                op("pe", pe_tail)
                s_mm.pinc()
                W("act", s_mm, s_mm.n)

                def act_tail(eng, ch=ch, bank=bank2):
                    s_x1.inc(eng.activation(out=xT[0:32, ch * 512:(ch + 1) * 512],
                                            in_=bank[0:32, :], func=ACOPY))
                op("act", act_tail)
                s_x1.pinc()
        tp_done_r[1] = s_tp.n

        if layer == 1:
            S0 = s_sh.n
            X1 = X0
            for ch in range(NS):
                W("pe", s_x1, X1 + ch + 1)
                if ch >= 1:
                    W("pe", s_sh, S0 + ch)
                if ch == NS - 1 and cfg.shard_real < SP:
                    W("dve", s_x1, X1 + NS)
                    op("dve", lambda eng: s_dve.inc(eng.memset(xT[0:32, cfg.shard_real:SP], 0)))
                    s_dve.pinc()
                    W("pe", s_dve, s_dve.n)

                def pe_sh(eng, ch=ch):
                    last = None
                    for t in range(4):
                        last = nc.tensor.transpose(
                            out=pbx[:, t * 32:(t + 1) * 32],
                            in_=xT[0:32, ch * 512 + t * 128:ch * 512 + (t + 1) * 128],
                            identity=sb_id32[:])
                    s_tp.inc(last)
                op("pe", pe_sh)
                s_tp.pinc()
                W("act", s_tp, s_tp.n)
                if ch == 0:
                    W("act", s_gp, gp_shard_done)   # shard DMA of this layer done

                def act_sh2(eng, ch=ch):
                    s_sh.inc(eng.activation(
                        out=sb_shard[:, 4 * ch:4 * ch + 4, :].rearrange("p a b -> p (a b)"),
                        in_=pbx[:, 0:128], func=ACOPY))
                op("act", act_sh2)
                s_sh.pinc()

    emit_layer(1)
    emit_layer(2)

    # =======================================================
    # head
    # =======================================================
    W("pe", s_x1, s_x1.n)
    XH = s_x1.n
    GH = s_gp.n
    for ch in range(NS):
        bank = pb[3 + (ch % 2)]
        if ch >= 1:
            W("pe", s_x1, XH + 2 * ch)    # act_h2(ch-1) consumed bank... (2 acts/ch)

        def pe_h1(eng, ch=ch, bank=bank):
            s_mm.inc(nc.tensor.matmul(bank[0:32, :], sb_wsm[:, 4, :],
                                      xT[0:33, ch * 512:(ch + 1) * 512], start=True, stop=True))
        op("pe", pe_h1)
        s_mm.pinc()
        W("act", s_mm, s_mm.n)
        op("act", lambda eng, bank=bank: s_x1.inc(eng.activation(
            out=sb_x3T[0:32, :], in_=bank[0:32, :], func=LRELU, alpha=0.01)))
        s_x1.pinc()
        W("pe", s_x1, s_x1.n)

        def pe_h2(eng, ch=ch, bank=bank):
            s_mm.inc(nc.tensor.matmul(bank[0:2, :], sb_wsm[:, 5, 0:2],
                                      sb_x3T[0:33, :], start=True, stop=True))
        op("pe", pe_h2)
        s_mm.pinc()
        W("act", s_mm, s_mm.n)
        if ch >= 2:
            W("act", s_gp, GH + (ch - 1) * 16)

        def act_h2(eng, ch=ch, bank=bank):
            s_x1.inc(eng.activation(out=sb_lg[:, ch % 2, :], in_=bank[0:2, :],
                                    func=ACOPY))
        op("act", act_h2)
        s_x1.pinc()
        W("sync", s_x1, s_x1.n)

        def gp_out(eng, ch=ch):
            s_gp.inc(eng.dma_start(out=out_ext[:, ch * 512:(ch + 1) * 512],
                                   in_=sb_lg[:, ch % 2, :]), 16)
        op("sync", gp_out)
        s_gp.pinc(16)
    W("sync", s_gp, s_gp.n)
    W("gp", s_gp, s_gp.n)

    with nc.Block() as block:
        @block.sync
        def _(eng):
            for f in plan["sync"]:
                f(eng)

        @block.tensor
        def _(eng):
            for f in plan["pe"]:
                f(eng)

        @block.scalar
        def _(eng):
            for f in plan["act"]:
                f(eng)

        @block.vector
        def _(eng):
            for f in plan["dve"]:
                f(eng)

        @block.gpsimd
        def _(eng):
            for f in plan["gp"]:
                f(eng)

    nc.compile()
    nc._live_refs = (live, mmctx)
    return nc


# =======================================================
# Host side
# =======================================================
def _build_structures(edge_index, edge_type, shard_real=12500, shard_pad=12800):
    SP = shard_pad
    src = edge_index[0].astype(np.int64)
    dst = edge_index[1].astype(np.int64)
    et = edge_type.astype(np.int64)
    owner = dst // shard_real
    ldst = dst % shard_real

    # sigma0: per-core node order = sorted by rel-0 in-degree (desc, stable).
    # All per-core node-indexed data (phase A inputs, table rows, output) live
    # in this order; rel-0's ELL sort then becomes the identity (no unpermute).
    perm0s, rank0s = [], []
    for c in range(N_CORES):
        d0 = np.bincount(ldst[(owner == c) & (et == 0)], minlength=SP)
        p0 = np.argsort(-d0, kind="stable")
        r0 = np.empty(SP, dtype=np.int64)
        r0[p0] = np.arange(SP)
        perm0s.append(p0)
        rank0s.append(r0)

    src_owner = src // shard_real
    src_local = src % shard_real
    all_rank0 = np.stack(rank0s)  # [N_CORES, SP]
    trow = src_owner * SP + all_rank0[src_owner, src_local]
    ldst = all_rank0[owner, ldst]  # relabel dst into sigma0 positions

    per_core = []
    for c in range(N_CORES):
        rels = []
        for r in range(NUM_REL):
            sel = (owner == c) & (et == r)
            l = ldst[sel]
            t = trow[sel]
            dcnt = np.bincount(l, minlength=SP)
            perm = np.argsort(-dcnt, kind="stable")
            rank = np.empty(SP, dtype=np.int64)
            rank[perm] = np.arange(SP)
            order = np.argsort(rank[l], kind="stable")
            l_s, t_s = l[order], t[order]
            s_sorted = rank[l_s]
            if len(l_s):
                newgrp = np.r_[True, s_sorted[1:] != s_sorted[:-1]]
                gidx = np.cumsum(newgrp) - 1
                starts = np.flatnonzero(newgrp)
                kpos = np.arange(len(l_s)) - starts[gidx]
            else:
                kpos = np.zeros(0, dtype=np.int64)
            maxd = int(dcnt.max()) if len(l) else 0
            Lk = np.array([(dcnt > k).sum() for k in range(maxd)], dtype=np.int64)
            rels.append(dict(dcnt=dcnt, perm=perm, rank=rank, s=s_sorted, k=kpos,
                             t=t_s, maxd=maxd, Lk=Lk))
        per_core.append(rels)

    maxd_g = [max(per_core[c][r]["maxd"] for c in range(N_CORES)) for r in range(NUM_REL)]
    c_r = []
    for r in range(NUM_REL):
        cks = []
        for k in range(maxd_g[r]):
            m = 1
            for c in range(N_CORES):
                Lk = per_core[c][r]["Lk"]
                if k < len(Lk):
                    m = max(m, int(np.ceil(Lk[k] / 128)))
            cks.append(m)
        c_r.append(cks)

    # piece decomposition (shared across cores)
    pieces = []
    colbase_r = []
    gcol = 0
    for r in range(NUM_REL):
        colbase = []
        plist = []
        cur_c0 = gcol
        cur_ck = 0
        cur_adds = []
        for k, ck in enumerate(c_r[r]):
            colbase.append(gcol)
            # split plane k into runs that fit the piece
            off = 0
            while off < ck:
                room = CBUF - cur_ck
                if room == 0:
                    plist.append((cur_c0, cur_ck, cur_adds))
                    cur_c0, cur_ck, cur_adds = cur_c0 + CBUF, 0, []
                    room = CBUF
                take = min(room, ck - off)
                cur_adds.append((off, cur_ck, take))
                cur_ck += take
                off += take
            gcol += ck
        if cur_ck:
            plist.append((cur_c0, cur_ck, cur_adds))
        pieces.append(plist)
        colbase_r.append(np.array(colbase, dtype=np.int64))

    cfg = Cfg(shard_real, SP, pieces)
    return cfg, per_core, colbase_r, gcol, perm0s


def _prep(inputs, shard_real=12500, shard_pad=12800):
    SP = shard_pad
    cfg, per_core, colbase_r, n_gcols, perm0s = _build_structures(
        inputs["edge_index"], inputs["edge_type"], shard_real, shard_pad)
    NT = cfg.nt
    CUNP = cfg.c_unp
    CTOT = n_gcols + 2 * CUNP

    f32 = np.float32
    bf16 = ml_dtypes.bfloat16
    wd = np.zeros((768, 32), dtype=bf16); wd[:, 0:8] = inputs["Wd"].astype(bf16)
    wt = np.zeros((768, 32), dtype=bf16); wt[:, 8:16] = inputs["Wt"].astype(bf16)
    wnc = np.zeros((16, 32), dtype=bf16)
    wnc[0:6, 16:24] = inputs["Wn"].astype(bf16)
    wnc[6:9, 24:32] = inputs["Wc"].astype(bf16)
    bx = np.zeros(32, dtype=np.float32)
    bx[0:8] = inputs["bd"]; bx[8:16] = inputs["bt"]
    bx[16:24] = inputs["bn"]; bx[24:32] = inputs["bc"]
    wnc[9, :] = bx.astype(bf16)
    wsm = np.zeros((6, 33, 32), dtype=bf16)
    wsm[0, 0:32] = inputs["Wi"].astype(bf16)
    wsm[0, 32] = inputs["bi"].astype(bf16)
    wsm[1, 0:32] = inputs["Wroot"].astype(bf16)
    wsm[1, 32] = inputs["brgcn"].astype(bf16)
    wsm[2, 0:32] = inputs["Wrel"][0].astype(bf16)
    wsm[3, 0:32] = inputs["Wrel"][1].astype(bf16)
    wsm[4, 0:32] = inputs["Wo1"].astype(bf16)
    wsm[4, 32] = inputs["bo1"].astype(bf16)
    wsm[5, 0:32, 0:2] = inputs["Wo2"].astype(bf16)
    wsm[5, 32, 0:2] = inputs["bo2"].astype(bf16)
    id128 = np.eye(128, dtype=f32)
    id32 = np.eye(32, dtype=bf16)

    in_maps = []
    for c in range(N_CORES):
        r0, r1 = c * shard_real, (c + 1) * shard_real
        p0 = perm0s[c]
        sel = p0[p0 < shard_real]      # sigma0 positions 0..shard_real-1 are real

        def _ft(full):  # [real, 768] f32 -> [128, 6*SP] bf16 feature-major
            a = np.zeros((SP, 768), dtype=bf16)
            a[0:shard_real] = full[r0:r1][sel].astype(bf16)
            # (p, c, n) = a[n, c*128 + p]
            return np.ascontiguousarray(
                a.T.reshape(6, 128, SP).transpose(1, 0, 2)).reshape(128, 6 * SP)

        desT = _ft(inputs["des"])
        twT = _ft(inputs["tweet"])
        nct = np.zeros((16, SP), dtype=bf16)
        nct[0:6, 0:shard_real] = inputs["num_prop"][r0:r1][sel].astype(bf16).T
        nct[6:9, 0:shard_real] = inputs["cat_prop"][r0:r1][sel].astype(bf16).T
        nct[9, 0:shard_real] = 1.0
        idx = np.full((128, CTOT), c * SP + SP - 1, dtype=np.int32)
        for r in range(NUM_REL):
            d = per_core[c][r]
            if len(d["s"]):
                jcol = d["s"] // 128
                p = d["s"] % 128
                cols = colbase_r[r][d["k"]] + jcol
                idx[p, cols] = d["t"].astype(np.int32)
        for r in range(NUM_REL):
            d = per_core[c][r]
            ucol = n_gcols + r * CUNP
            n = np.arange(SP)
            idx[n % 128, ucol + n // 128] = d["rank"][n].astype(np.int32)
        coefnm = np.zeros((128, NUM_REL, NT), dtype=f32)
        for r in range(NUM_REL):
            d = per_core[c][r]
            cv = (1.0 / np.maximum(d["dcnt"][d["perm"]], 1)).astype(f32)  # sorted order
            coefnm[:, r, :] = cv.reshape(NT, 128).T
        in_maps.append({
            "desT": desT, "twT": twT, "nct": nct, "idx": idx, "coefnm": coefnm,
            "wd": wd, "wt": wt, "wnc": wnc, "wsm": wsm,
            "id128": id128, "id32": id32,
        })
    return cfg, in_maps, perm0s


_CACHE = {}
LAST_RESULT = None


def kernel(**inputs):
    global LAST_RESULT
    cfg, in_maps, perm0s = _prep(inputs)
    key = tuple((c0, ck) for r in range(NUM_REL) for (c0, ck, _) in cfg.pieces[r])
    if key not in _CACHE:
        _CACHE[key] = build_bass(cfg)
    nc = _CACHE[key]
    res = run_bass_kernel_spmd(nc, in_maps, list(range(N_CORES)))
    LAST_RESULT = res
    outs = []
    for c in range(N_CORES):
        o = res.results[c]["out"].T  # [SP, 2], rows in sigma0 order
        full = np.empty((cfg.shard_real, 2), dtype=np.float32)
        p0 = perm0s[c]
        sel = p0[p0 < cfg.shard_real]
        full[sel] = o[0:cfg.shard_real]
        outs.append(full)
    return np.ascontiguousarray(np.concatenate(outs, axis=0).astype(np.float32))



# revision 18
# speedup vs baseline: 1.0024x; 1.0024x over previous
"""BotRGCN on 8 trn2 NeuronCores (SPMD, raw Bacc).

Nodes row-sharded 8 ways (12500/core, padded to 12800), each core's
shard relabelled into sigma0 order (sorted by rel-0 in-degree) so
rel-0's degree-sorted ELL needs no unpermute. Phase A consumes
host-pretransposed bf16 feature-major inputs ([128, 6*SP]) in a
software-pipelined mm/Wi/transpose schedule with ACT-fused leaky-relu.
RGCN layers: AllGather bf16 node features -> shared gather table;
per-relation ELL plane gathers via indirect DMA (int32 idx, 64B rows,
~1.1us/128 rows on the Q7 -- the kernel's floor); DVE accumulate +
1/cnt scale into per-relation agg buffers; rel-1 unpermuted via a DRAM
round trip interleaved chunk-by-chunk with its transposes and the
dense tail; shard/scratch DMAs ride the idle sync engine.
"""
import sys
sys.path.insert(0, "/opt/trn_rl_repo")
from contextlib import ExitStack

import numpy as np
import ml_dtypes

from concourse import bacc, bass, mybir
from concourse import library_config
from concourse.bass_utils import run_bass_kernel_spmd

F32 = mybir.dt.float32
BF16 = mybir.dt.bfloat16
I32 = mybir.dt.int32
LRELU = mybir.ActivationFunctionType.Lrelu
ACOPY = mybir.ActivationFunctionType.Copy

N_CORES = 8
NUM_REL = 2
CBUF = 96          # gather piece size (idx columns per indirect DMA)


class Cfg:
    def __init__(self, shard_real, shard_pad, pieces):
        self.shard_real = shard_real
        self.shard_pad = shard_pad
        self.n_super = shard_pad // 512
        self.nt = shard_pad // 128
        self.c_unp = shard_pad // 128
        self.tabv = N_CORES * shard_pad
        # pieces[r] = list of (idx_col0, ck, adds) ; adds = [(agg_blk0, msg_blk0, nblk)]
        self.pieces = pieces
        self.ctot = (sum(ck for p in pieces for (_, ck, _) in [p[1]] for p in [p]) if False
                     else None)


def build_bass(cfg: Cfg):
    nc = bacc.Bacc("TRN2", debug=False)
    mmctx = ExitStack()
    SP = cfg.shard_pad
    NT = cfg.nt
    NS = cfg.n_super
    TABV = cfg.tabv
    CUNP = cfg.c_unp
    n_gcols = max(c0 + ck for r in range(NUM_REL) for (c0, ck, _) in cfg.pieces[r]) \
        if any(cfg.pieces) else 0
    CTOT = n_gcols + 2 * CUNP

    desT_in = nc.declare_dram_parameter("desT", [128, 6 * SP], BF16, isOutput=False)
    twT_in = nc.declare_dram_parameter("twT", [128, 6 * SP], BF16, isOutput=False)
    nct_in = nc.declare_dram_parameter("nct", [16, SP], BF16, isOutput=False)
    idx_in = nc.declare_dram_parameter("idx", [128, CTOT], I32, isOutput=False)
    coef_in = nc.declare_dram_parameter("coefnm", [128, NUM_REL, NT], F32, isOutput=False)
    wd_in = nc.declare_dram_parameter("wd", [768, 32], BF16, isOutput=False)
    wt_in = nc.declare_dram_parameter("wt", [768, 32], BF16, isOutput=False)
    wnc_in = nc.declare_dram_parameter("wnc", [16, 32], BF16, isOutput=False)
    wsm_in = nc.declare_dram_parameter("wsm", [6, 33, 32], BF16, isOutput=False)
    id128_in = nc.declare_dram_parameter("id128", [128, 128], F32, isOutput=False)
    id32_in = nc.declare_dram_parameter("id32", [32, 32], BF16, isOutput=False)
    out_ext = nc.declare_dram_parameter("out", [2, SP], F32, isOutput=True)

    shard_ag = nc.dram_tensor("shard_ag", [SP, 32], BF16)
    table = nc.dram_tensor("table", [TABV, 32], BF16, addr_space="Shared")
    scratch = nc.dram_tensor("scratch", [SP, 32], F32)

    live = []

    def sb(name, shape, dt):
        cm = nc.sbuf_tensor(name, shape, dt)
        t = cm.__enter__()
        live.append(cm)
        return t

    def psum_dt(name, shape, dt):
        cm = nc.psum_tensor(name, shape, dt)
        t = cm.__enter__()
        live.append(cm)
        return t

    def psum(name, shape):
        return psum_dt(name, shape, F32)

    sb_dT = sb("sb_dT", [128, 2, 6, 512], BF16)
    sb_tT = sb("sb_tT", [128, 2, 6, 512], BF16)
    sb_ncT = sb("sb_ncT", [16, SP], BF16)
    xT = sb("xT", [33, SP], BF16)
    aggT0 = sb("aggT0", [32, SP], BF16)
    aggT1 = sb("aggT1", [32, SP], BF16)
    aggb = [sb("agg0", [128, NT * 32], F32), sb("agg1", [128, NT * 32], F32)]
    sb("pad_align", [128, 3584], mybir.dt.uint8)
    msgs2 = [sb("msgsA", [128, CBUF * 32], BF16), sb("msgsB", [128, CBUF * 32], BF16)]
    sb_shard = sb("sb_shard", [128, NT, 32], BF16)
    sb_idxg = [[sb(f"sb_idxg{r}_{pi}", [128, ck], I32)
                for pi, (c0, ck, _) in enumerate(cfg.pieces[r])] for r in range(NUM_REL)]
    sb_idxu = [sb(f"sb_idxu{r}", [128, CUNP], I32) for r in range(NUM_REL)]
    sb_coef = sb("sb_coef", [128, NUM_REL, NT], F32)
    sb_wd = sb("sb_wd", [128, 6, 32], BF16)
    sb_wt = sb("sb_wt", [128, 6, 32], BF16)
    sb_wnc = sb("sb_wnc", [16, 32], BF16)
    sb_wsm = sb("sb_wsm", [33, 6, 32], BF16)
    sb_id128 = sb("sb_id128", [128, 128], F32)
    sb_id32 = sb("sb_id32", [32, 32], BF16)
    sb_x3T = sb("sb_x3T", [33, 512], BF16)
    sb_lg = sb("sb_lg", [2, 2, 512], F32)

    pb = [psum(f"pb{i}", [128, 512]) for i in range(8)]
    pbx = pb[5][:, :].bitcast(BF16)

    plan = {"sync": [], "pe": [], "act": [], "dve": [], "gp": []}

    def op(engine, fn):
        plan[engine].append(fn)

    class Sem:
        def __init__(self, name):
            cm = nc.semaphore(name)
            self.h = cm.__enter__()
            live.append(cm)
            self.n = 0

        def inc(self, inst, k=1):
            # runtime half: attach the semaphore update (no counting here)
            inst.then_inc(self.h, k)

        def pinc(self, k=1):
            # plan-time half: advance the cumulative count
            self.n += k
            return self.n

    s_load = Sem("s_load")
    s_a1 = Sem("s_a1")
    s_ld = [Sem("s_ld0"), Sem("s_ld1")]
    s_lr = Sem("s_lr")
    s_gq = [Sem(f"s_gq{i}") for i in range(8)]
    s_tp = Sem("s_tp")
    s_cp = Sem("s_cp")
    s_mm = Sem("s_mm")
    s_x1 = Sem("s_x1")
    s_gp = Sem("s_gp")
    s_cc = Sem("s_cc")
    s_dve = Sem("s_dve")
    s_sh = Sem("s_sh")

    def W(engine, sem, val):
        if val > 0:
            op(engine, lambda eng, s=sem, v=val: eng.wait_ge(s.h, v))

    # ---------------- constants ----------------
    def c_loads(eng):
        for r in range(NUM_REL):
            for pi, (c0, ck, _) in enumerate(cfg.pieces[r]):
                eng.dma_start(out=sb_idxg[r][pi][:], in_=idx_in[:, c0:c0 + ck]).then_inc(s_load.h, 16)
            u0 = n_gcols + r * CUNP
            eng.dma_start(out=sb_idxu[r][:], in_=idx_in[:, u0:u0 + CUNP]).then_inc(s_load.h, 16)
        eng.dma_start(out=sb_coef[:], in_=coef_in[:, :, :]).then_inc(s_load.h, 16)
        eng.dma_start(out=sb_wd[:], in_=wd_in.ap().rearrange("(c p) m -> p c m", p=128)).then_inc(s_load.h, 16)
        eng.dma_start(out=sb_wt[:], in_=wt_in.ap().rearrange("(c p) m -> p c m", p=128)).then_inc(s_load.h, 16)
        eng.dma_start(out=sb_wnc[:], in_=wnc_in[:, :]).then_inc(s_load.h, 16)
        eng.dma_start(out=sb_wsm[:], in_=wsm_in.ap().rearrange("c p m -> p c m")).then_inc(s_load.h, 16)
        eng.dma_start(out=sb_ncT[:], in_=nct_in[:, :]).then_inc(s_load.h, 16)
        eng.dma_start(out=sb_id128[:], in_=id128_in[:, :]).then_inc(s_load.h, 16)
        eng.dma_start(out=sb_id32[:], in_=id32_in[:, :]).then_inc(s_load.h, 16)
    op("sync", c_loads)
    s_load.n += (8 + sum(len(p) for p in cfg.pieces) + NUM_REL) * 16
    NCONST = s_load.n

    op("gp", lambda eng: eng.load_library(library_config.mlp))

    def init_ones(eng):
        eng.memset(xT[32:33, :], 1.0)
        s_dve.inc(eng.memset(sb_x3T[32:33, :], 1.0))
    op("dve", init_ones)
    s_dve.pinc()
    NINIT = s_dve.n

    # =======================================================
    # Phase A: xT = lrelu(lrelu([d|t|n|c]) @ Wi + bi), inputs pre-transposed bf16
    # =======================================================
    # Software-pipelined: step v runs mm(v) | wi(v-1) | x1t(v-2) on PE and
    # a1(v) | a2(v-1) | act_sh(v-2) on ACT. pb6 rotates over 3 banks, pb7 over 2.
    mm_no = {}
    wi_no = {}
    for v in range(NS + 2):
        if v < NS:
            buf = v % 2

            def ld(eng, v=v, buf=buf):
                eng.dma_start(
                    out=sb_dT[:, buf, :, :],
                    in_=desT_in.ap().rearrange("p (c n) -> p c n", n=SP)[:, :, v * 512:(v + 1) * 512],
                ).then_inc(s_ld[buf].h, 16)
                eng.dma_start(
                    out=sb_tT[:, buf, :, :],
                    in_=twT_in.ap().rearrange("p (c n) -> p c n", n=SP)[:, :, v * 512:(v + 1) * 512],
                ).then_inc(s_ld[buf].h, 16)
            if v == 2:
                W("sync", s_mm, 1)
            elif v >= 3:
                W("sync", s_mm, 2 * v - 4)
            op("sync", ld)
            s_ld[buf].n += 32

        # ---- PE ----
        if v < NS:
            if v == 0:
                W("pe", s_load, NCONST)
                W("pe", s_dve, NINIT)
            W("pe", s_ld[v % 2], 32 * (v // 2 + 1))
            if v >= 3:
                W("pe", s_a1, v - 2)       # a1(v-3) freed this pb6 bank
            b6 = pb[v % 3]

            def pe_mm(eng, v=v, b6=b6):
                for c in range(6):
                    nc.tensor.matmul(b6[0:32, :], sb_wd[:, c, :], sb_dT[:, v % 2, c, :],
                                     start=(c == 0), stop=False)
                for c in range(6):
                    nc.tensor.matmul(b6[0:32, :], sb_wt[:, c, :], sb_tT[:, v % 2, c, :],
                                     start=False, stop=False)
                last = nc.tensor.matmul(b6[0:32, :], sb_wnc[:, :],
                                        sb_ncT[:, v * 512:(v + 1) * 512],
                                        start=False, stop=True)
                s_mm.inc(last)
            op("pe", pe_mm)
            mm_no[v] = s_mm.pinc()

        if v >= 1 and v - 1 < NS:
            W("pe", s_a1, v)               # a1(v-1) produced xT slice
            if v >= 3:
                W("pe", s_x1, v - 2)       # a2(v-3) freed this pb7 bank
            b7 = pb[3 + (v - 1) % 2]

            def pe_wi(eng, v=v, b7=b7):
                s_mm.inc(nc.tensor.matmul(b7[0:32, :], sb_wsm[:, 0, :],
                                          xT[0:33, (v - 1) * 512:v * 512],
                                          start=True, stop=True))
            op("pe", pe_wi)
            wi_no[v - 1] = s_mm.pinc()

        if v >= 2 and v - 2 < NS:
            i = v - 2
            if i == NS - 1 and cfg.shard_real < SP:
                W("dve", s_x1, NS)
                op("dve", lambda eng: s_dve.inc(eng.memset(xT[0:32, cfg.shard_real:SP], 0)))
                s_dve.pinc()
                W("pe", s_dve, NINIT + 1)
            W("pe", s_x1, i + 1)           # a2(i) done
            if i >= 1:
                W("pe", s_sh, i)           # act_sh(i-1) freed pbx

            def pe_x1t(eng, i=i):
                last = None
                for t in range(4):
                    last = nc.tensor.transpose(
                        out=pbx[:, t * 32:(t + 1) * 32],
                        in_=xT[0:32, i * 512 + t * 128:i * 512 + (t + 1) * 128],
                        identity=sb_id32[:])
                s_tp.inc(last)
            op("pe", pe_x1t)
            s_tp.pinc()

        # ---- ACT ----
        if v < NS:
            W("act", s_mm, mm_no[v])

            def a1(eng, v=v, b6=pb[v % 3]):
                s_a1.inc(eng.activation(out=xT[0:32, v * 512:(v + 1) * 512],
                                        in_=b6[0:32, :], func=LRELU, alpha=0.01))
            op("act", a1)
            s_a1.pinc()

        if v >= 1 and v - 1 < NS:
            W("act", s_mm, wi_no[v - 1])

            def a2(eng, v=v, b7=pb[3 + (v - 1) % 2]):
                s_x1.inc(eng.activation(out=xT[0:32, (v - 1) * 512:v * 512],
                                        in_=b7[0:32, :], func=LRELU, alpha=0.01))
            op("act", a2)
            s_x1.pinc()

        if v >= 2 and v - 2 < NS:
            i = v - 2
            W("act", s_tp, i + 1)

            def act_sh(eng, i=i):
                s_sh.inc(eng.activation(
                    out=sb_shard[:, 4 * i:4 * i + 4, :].rearrange("p a b -> p (a b)"),
                    in_=pbx[:, 0:128], func=ACOPY))
            op("act", act_sh)
            s_sh.pinc()

    # =======================================================
    # RGCN layers
    # =======================================================
    gq_rr = [0]
    unp_done = [0]
    tp_done_r = {0: 0, 1: 0}

    def emit_layer(layer):
        # shard -> shard_ag in 5 chunks on the (idle) sync engine, each fired
        # as soon as its act_sh slices are done -- overlaps the tail compute
        S_base = s_sh.n - NS      # act_sh counts for this layer start here
        if layer == 2:
            W("sync", s_cc, s_cc.n)   # AllGather-1 consumed shard_ag
        for chk in range(5):
            t0, t1 = chk * (NT // 5), (chk + 1) * (NT // 5)
            W("sync", s_sh, S_base + (t1 // 4))

            def shard_chunk(eng, t0=t0, t1=t1):
                s_gp.inc(eng.dma_start(
                    out=shard_ag[:, :].rearrange("(t p) d -> p t d", p=128)[:, t0:t1, :],
                    in_=sb_shard[:, t0:t1, :]), 16)
            op("sync", shard_chunk)
            s_gp.pinc(16)
        gp_shard_done = s_gp.n
        W("gp", s_gp, gp_shard_done)
        op("gp", lambda eng: s_cc.inc(eng.collective_compute(
            "AllGather", mybir.AluOpType.bypass,
            ins=[shard_ag[:, :]], outs=[table[:, :]],
            replica_groups=[list(range(N_CORES))])))
        s_cc.pinc()
        W("gp", s_cc, s_cc.n)

        # hoisted memsets: each relation's agg buffer cleared as soon as its
        # previous-layer transposes have consumed it
        D0r = {}
        for r in range(NUM_REL):
            W("dve", s_tp, tp_done_r[r])
            op("dve", lambda eng, r=r: s_dve.inc(eng.memset(aggb[r][:, :], 0)))
            D0r[r] = s_dve.pinc()

        for r in range(NUM_REL):
            ag = aggb[r]
            # accs for this relation start at inc D0+1 (both memsets precede r0's)
            D0 = D0r[1] if r == 0 else s_dve.n
            for pi, (c0, ck, adds) in enumerate(cfg.pieces[r]):
                pbuf = pi % 2
                sq = (gq_rr[0]) % 8
                gq_rr[0] += 1
                # gather pi waits DVE finished consuming msgs[pbuf] (piece pi-2)
                W("gp", s_dve, D0 + pi - 1 if pi >= 2 else D0)

                def gth(eng, r=r, pi=pi, ck=ck, pbuf=pbuf, sq=sq):
                    for j in range(ck):
                        inst = eng.indirect_dma_start(
                            out=msgs2[pbuf][:, j * 32:(j + 1) * 32],
                            out_offset=None,
                            in_=table[:, :],
                            in_offset=bass.IndirectOffsetOnAxis(
                                ap=sb_idxg[r][pi][:, j:j + 1], axis=0),
                        )
                        s_gq[sq].inc(inst, 16)
                op("gp", gth)
                s_gq[sq].pinc(16 * ck)
                W("dve", s_gq[sq], s_gq[sq].n)

                def acc(eng, adds=adds, pbuf=pbuf, ag=ag):
                    last = None
                    for j, (ab, mb, nb) in enumerate(adds):
                        if j > 0:
                            eng.drain()
                        last = nc.vector.tensor_tensor(
                            out=ag[:, ab * 32:(ab + nb) * 32],
                            in0=ag[:, ab * 32:(ab + nb) * 32],
                            in1=msgs2[pbuf][:, mb * 32:(mb + nb) * 32],
                            op=mybir.AluOpType.add)
                    s_dve.inc(last)
                op("dve", acc)
                s_dve.pinc()

            # scale by 1/cnt (sorted order), per node-tile
            def scl(eng, r=r, ag=ag):
                eng.drain()
                last = None
                for t in range(NT):
                    last = nc.vector.tensor_tensor(
                        out=ag[:, t * 32:(t + 1) * 32],
                        in0=ag[:, t * 32:(t + 1) * 32],
                        in1=sb_coef[:, r, t:t + 1].to_broadcast([128, 32]),
                        op=mybir.AluOpType.mult)
                s_dve.inc(last)
            op("dve", scl)
            s_dve.pinc()

            if r == 0:
                # r0 is already canonical: transpose agg0 -> aggT0 right away
                W("pe", s_dve, s_dve.n)
                C0 = s_cp.n
                T0 = s_tp.n
                for g in range(NT // 4):
                    bank = pb[1 + (g % 2)]
                    W("pe", s_cp, C0 + g - 1 if g >= 2 else C0)

                    def pe_at(eng, g=g, bank=bank, ag=ag):
                        last = None
                        for t in range(4):
                            n = g * 4 + t
                            last = nc.tensor.transpose(out=bank[0:32, t * 128:(t + 1) * 128],
                                                       in_=ag[:, n * 32:(n + 1) * 32],
                                                       identity=sb_id128[:])
                        s_tp.inc(last)
                    op("pe", pe_at)
                    s_tp.pinc()
                    W("act", s_tp, T0 + g + 1)

                    def act_at(eng, g=g, bank=bank):
                        s_cp.inc(eng.activation(out=aggT0[:, g * 512:(g + 1) * 512],
                                                in_=bank[0:32, :], func=ACOPY))
                    op("act", act_at)
                    s_cp.pinc()
                tp_done_r[0] = s_tp.n

        # r1: scratch round trip (sorted -> sigma0), then interleaved
        # unp -> transpose -> tail, chunk by chunk
        if layer == 2:
            W("sync", s_gp, unp_done[0])   # layer-1 unp consumed scratch
        W("sync", s_dve, s_dve.n)
        op("sync", lambda eng: s_gp.inc(
            eng.dma_start(out=scratch[:, :].rearrange("(t p) d -> p t d", p=128),
                          in_=aggb[1][:, :].rearrange("p (t d) -> p t d", d=32)), 16))
        s_gp.pinc(16)
        W("gp", s_gp, s_gp.n)
        U0 = s_gp.n

        def unp(eng):
            for j in range(CUNP):
                inst = eng.indirect_dma_start(
                    out=aggb[1][:, j * 32:(j + 1) * 32],
                    out_offset=None,
                    in_=scratch[:, :],
                    in_offset=bass.IndirectOffsetOnAxis(ap=sb_idxu[1][:, j:j + 1], axis=0),
                )
                s_gp.inc(inst, 16)
        op("gp", unp)
        s_gp.pinc(16 * CUNP)
        unp_done[0] = s_gp.n

        cp_r1 = s_cp.n
        T0 = s_tp.n
        X0 = s_x1.n
        for g in range(NT // 4 + 1):
            if g < NT // 4:
                bank = pb[1 + (g % 2)]
                W("pe", s_cp, cp_r1 + g - 1 if g >= 2 else cp_r1)
                W("pe", s_gp, U0 + 16 * (4 * g + 4))

                def pe_at1(eng, g=g, bank=bank):
                    last = None
                    for t in range(4):
                        n = g * 4 + t
                        last = nc.tensor.transpose(out=bank[0:32, t * 128:(t + 1) * 128],
                                                   in_=aggb[1][:, n * 32:(n + 1) * 32],
                                                   identity=sb_id128[:])
                    s_tp.inc(last)
                op("pe", pe_at1)
                s_tp.pinc()
                W("act", s_tp, T0 + g + 1)

                def act_at1(eng, g=g, bank=bank):
                    s_cp.inc(eng.activation(out=aggT1[:, g * 512:(g + 1) * 512],
                                            in_=bank[0:32, :], func=ACOPY))
                op("act", act_at1)
                s_cp.pinc()
            if g >= 1:
                ch = g - 1
                bank2 = pb[3 + (ch % 2)]
                if ch >= 2:
                    W("pe", s_x1, X0 + ch - 1)
                else:
                    W("pe", s_x1, X0)
                W("pe", s_cp, cp_r1 + ch + 1)

                def pe_tail(eng, ch=ch, bank=bank2):
                    sl = slice(ch * 512, (ch + 1) * 512)
                    nc.tensor.matmul(bank[0:32, :], sb_wsm[:, 1, :], xT[0:33, sl],
                                     start=True, stop=False)
                    nc.tensor.matmul(bank[0:32, :], sb_wsm[0:32, 2, :], aggT0[:, sl],
                                     start=False, stop=False)
                    last = nc.tensor.matmul(bank[0:32, :], sb_wsm[0:32, 3, :], aggT1[:, sl],
                                            start=False, stop=True)
                    s_mm.inc(last)
                op("pe", pe_tail)
                s_mm.pinc()
                W("act", s_mm, s_mm.n)

                def act_tail(eng, ch=ch, bank=bank2):
                    s_x1.inc(eng.activation(out=xT[0:32, ch * 512:(ch + 1) * 512],
                                            in_=bank[0:32, :], func=ACOPY))
                op("act", act_tail)
                s_x1.pinc()
        tp_done_r[1] = s_tp.n

        if layer == 1:
            S0 = s_sh.n
            X1 = X0
            for ch in range(NS):
                W("pe", s_x1, X1 + ch + 1)
                if ch >= 1:
                    W("pe", s_sh, S0 + ch)
                if ch == NS - 1 and cfg.shard_real < SP:
                    W("dve", s_x1, X1 + NS)
                    op("dve", lambda eng: s_dve.inc(eng.memset(xT[0:32, cfg.shard_real:SP], 0)))
                    s_dve.pinc()
                    W("pe", s_dve, s_dve.n)

                def pe_sh(eng, ch=ch):
                    last = None
                    for t in range(4):
                        last = nc.tensor.transpose(
                            out=pbx[:, t * 32:(t + 1) * 32],
                            in_=xT[0:32, ch * 512 + t * 128:ch * 512 + (t + 1) * 128],
                            identity=sb_id32[:])
                    s_tp.inc(last)
                op("pe", pe_sh)
                s_tp.pinc()
                W("act", s_tp, s_tp.n)
                if ch == 0:
                    W("act", s_gp, gp_shard_done)   # shard DMA of this layer done

                def act_sh2(eng, ch=ch):
                    s_sh.inc(eng.activation(
                        out=sb_shard[:, 4 * ch:4 * ch + 4, :].rearrange("p a b -> p (a b)"),
                        in_=pbx[:, 0:128], func=ACOPY))
                op("act", act_sh2)
                s_sh.pinc()

    emit_layer(1)
    emit_layer(2)

    # =======================================================
    # head
    # =======================================================
    W("pe", s_x1, s_x1.n)
    XH = s_x1.n
    GH = s_gp.n
    for ch in range(NS):
        bank = pb[3 + (ch % 2)]
        if ch >= 1:
            W("pe", s_x1, XH + 2 * ch)    # act_h2(ch-1) consumed bank... (2 acts/ch)

        def pe_h1(eng, ch=ch, bank=bank):
            s_mm.inc(nc.tensor.matmul(bank[0:32, :], sb_wsm[:, 4, :],
                                      xT[0:33, ch * 512:(ch + 1) * 512], start=True, stop=True))
        op("pe", pe_h1)
        s_mm.pinc()
        W("act", s_mm, s_mm.n)
        op("act", lambda eng, bank=bank: s_x1.inc(eng.activation(
            out=sb_x3T[0:32, :], in_=bank[0:32, :], func=LRELU, alpha=0.01)))
        s_x1.pinc()
        W("pe", s_x1, s_x1.n)

        def pe_h2(eng, ch=ch, bank=bank):
            s_mm.inc(nc.tensor.matmul(bank[0:2, :], sb_wsm[:, 5, 0:2],
                                      sb_x3T[0:33, :], start=True, stop=True))
        op("pe", pe_h2)
        s_mm.pinc()
        W("act", s_mm, s_mm.n)
        if ch >= 2:
            W("act", s_gp, GH + (ch - 1) * 16)

        def act_h2(eng, ch=ch, bank=bank):
            s_x1.inc(eng.activation(out=sb_lg[:, ch % 2, :], in_=bank[0:2, :],
                                    func=ACOPY))
        op("act", act_h2)
        s_x1.pinc()
        W("sync", s_x1, s_x1.n)

        def gp_out(eng, ch=ch):
            s_gp.inc(eng.dma_start(out=out_ext[:, ch * 512:(ch + 1) * 512],
                                   in_=sb_lg[:, ch % 2, :]), 16)
        op("sync", gp_out)
        s_gp.pinc(16)
    W("sync", s_gp, s_gp.n)
    W("gp", s_gp, s_gp.n)

    with nc.Block() as block:
        @block.sync
        def _(eng):
            for f in plan["sync"]:
                f(eng)

        @block.tensor
        def _(eng):
            for f in plan["pe"]:
                f(eng)

        @block.scalar
        def _(eng):
            for f in plan["act"]:
                f(eng)

        @block.vector
        def _(eng):
            for f in plan["dve"]:
                f(eng)

        @block.gpsimd
        def _(eng):
            for f in plan["gp"]:
                f(eng)

    nc.compile()
    nc._live_refs = (live, mmctx)
    return nc


# =======================================================
# Host side
# =======================================================
def _build_structures(edge_index, edge_type, shard_real=12500, shard_pad=12800):
    SP = shard_pad
    src = edge_index[0].astype(np.int64)
    dst = edge_index[1].astype(np.int64)
    et = edge_type.astype(np.int64)
    owner = dst // shard_real
    ldst = dst % shard_real

    # sigma0: per-core node order = sorted by rel-0 in-degree (desc, stable).
    # All per-core node-indexed data (phase A inputs, table rows, output) live
    # in this order; rel-0's ELL sort then becomes the identity (no unpermute).
    perm0s, rank0s = [], []
    for c in range(N_CORES):
        d0 = np.bincount(ldst[(owner == c) & (et == 0)], minlength=SP)
        p0 = np.argsort(-d0, kind="stable")
        r0 = np.empty(SP, dtype=np.int64)
        r0[p0] = np.arange(SP)
        perm0s.append(p0)
        rank0s.append(r0)

    src_owner = src // shard_real
    src_local = src % shard_real
    all_rank0 = np.stack(rank0s)  # [N_CORES, SP]
    trow = src_owner * SP + all_rank0[src_owner, src_local]
    ldst = all_rank0[owner, ldst]  # relabel dst into sigma0 positions

    per_core = []
    for c in range(N_CORES):
        rels = []
        for r in range(NUM_REL):
            sel = (owner == c) & (et == r)
            l = ldst[sel]
            t = trow[sel]
            dcnt = np.bincount(l, minlength=SP)
            perm = np.argsort(-dcnt, kind="stable")
            rank = np.empty(SP, dtype=np.int64)
            rank[perm] = np.arange(SP)
            order = np.argsort(rank[l], kind="stable")
            l_s, t_s = l[order], t[order]
            s_sorted = rank[l_s]
            if len(l_s):
                newgrp = np.r_[True, s_sorted[1:] != s_sorted[:-1]]
                gidx = np.cumsum(newgrp) - 1
                starts = np.flatnonzero(newgrp)
                kpos = np.arange(len(l_s)) - starts[gidx]
            else:
                kpos = np.zeros(0, dtype=np.int64)
            maxd = int(dcnt.max()) if len(l) else 0
            Lk = np.array([(dcnt > k).sum() for k in range(maxd)], dtype=np.int64)
            rels.append(dict(dcnt=dcnt, perm=perm, rank=rank, s=s_sorted, k=kpos,
                             t=t_s, maxd=maxd, Lk=Lk))
        per_core.append(rels)

    maxd_g = [max(per_core[c][r]["maxd"] for c in range(N_CORES)) for r in range(NUM_REL)]
    c_r = []
    for r in range(NUM_REL):
        cks = []
        for k in range(maxd_g[r]):
            m = 1
            for c in range(N_CORES):
                Lk = per_core[c][r]["Lk"]
                if k < len(Lk):
                    m = max(m, int(np.ceil(Lk[k] / 128)))
            cks.append(m)
        c_r.append(cks)

    # piece decomposition (shared across cores)
    pieces = []
    colbase_r = []
    gcol = 0
    for r in range(NUM_REL):
        colbase = []
        plist = []
        cur_c0 = gcol
        cur_ck = 0
        cur_adds = []
        for k, ck in enumerate(c_r[r]):
            colbase.append(gcol)
            # split plane k into runs that fit the piece
            off = 0
            while off < ck:
                room = CBUF - cur_ck
                if room == 0:
                    plist.append((cur_c0, cur_ck, cur_adds))
                    cur_c0, cur_ck, cur_adds = cur_c0 + CBUF, 0, []
                    room = CBUF
                take = min(room, ck - off)
                cur_adds.append((off, cur_ck, take))
                cur_ck += take
                off += take
            gcol += ck
        if cur_ck:
            plist.append((cur_c0, cur_ck, cur_adds))
        pieces.append(plist)
        colbase_r.append(np.array(colbase, dtype=np.int64))

    cfg = Cfg(shard_real, SP, pieces)
    return cfg, per_core, colbase_r, gcol, perm0s


def _prep(inputs, shard_real=12500, shard_pad=12800):
    SP = shard_pad
    cfg, per_core, colbase_r, n_gcols, perm0s = _build_structures(
        inputs["edge_index"], inputs["edge_type"], shard_real, shard_pad)
    NT = cfg.nt
    CUNP = cfg.c_unp
    CTOT = n_gcols + 2 * CUNP

    f32 = np.float32
    bf16 = ml_dtypes.bfloat16
    wd = np.zeros((768, 32), dtype=bf16); wd[:, 0:8] = inputs["Wd"].astype(bf16)
    wt = np.zeros((768, 32), dtype=bf16); wt[:, 8:16] = inputs["Wt"].astype(bf16)
    wnc = np.zeros((16, 32), dtype=bf16)
    wnc[0:6, 16:24] = inputs["Wn"].astype(bf16)
    wnc[6:9, 24:32] = inputs["Wc"].astype(bf16)
    bx = np.zeros(32, dtype=np.float32)
    bx[0:8] = inputs["bd"]; bx[8:16] = inputs["bt"]
    bx[16:24] = inputs["bn"]; bx[24:32] = inputs["bc"]
    wnc[9, :] = bx.astype(bf16)
    wsm = np.zeros((6, 33, 32), dtype=bf16)
    wsm[0, 0:32] = inputs["Wi"].astype(bf16)
    wsm[0, 32] = inputs["bi"].astype(bf16)
    wsm[1, 0:32] = inputs["Wroot"].astype(bf16)
    wsm[1, 32] = inputs["brgcn"].astype(bf16)
    wsm[2, 0:32] = inputs["Wrel"][0].astype(bf16)
    wsm[3, 0:32] = inputs["Wrel"][1].astype(bf16)
    wsm[4, 0:32] = inputs["Wo1"].astype(bf16)
    wsm[4, 32] = inputs["bo1"].astype(bf16)
    wsm[5, 0:32, 0:2] = inputs["Wo2"].astype(bf16)
    wsm[5, 32, 0:2] = inputs["bo2"].astype(bf16)
    id128 = np.eye(128, dtype=f32)
    id32 = np.eye(32, dtype=bf16)

    in_maps = []
    for c in range(N_CORES):
        r0, r1 = c * shard_real, (c + 1) * shard_real
        p0 = perm0s[c]
        sel = p0[p0 < shard_real]      # sigma0 positions 0..shard_real-1 are real

        def _ft(full):  # [real, 768] f32 -> [128, 6*SP] bf16 feature-major
            a = np.zeros((SP, 768), dtype=bf16)
            a[0:shard_real] = full[r0:r1][sel].astype(bf16)
            # (p, c, n) = a[n, c*128 + p]
            return np.ascontiguousarray(
                a.T.reshape(6, 128, SP).transpose(1, 0, 2)).reshape(128, 6 * SP)

        desT = _ft(inputs["des"])
        twT = _ft(inputs["tweet"])
        nct = np.zeros((16, SP), dtype=bf16)
        nct[0:6, 0:shard_real] = inputs["num_prop"][r0:r1][sel].astype(bf16).T
        nct[6:9, 0:shard_real] = inputs["cat_prop"][r0:r1][sel].astype(bf16).T
        nct[9, 0:shard_real] = 1.0
        idx = np.full((128, CTOT), c * SP + SP - 1, dtype=np.int32)
        for r in range(NUM_REL):
            d = per_core[c][r]
            if len(d["s"]):
                jcol = d["s"] // 128
                p = d["s"] % 128
                cols = colbase_r[r][d["k"]] + jcol
                idx[p, cols] = d["t"].astype(np.int32)
        for r in range(NUM_REL):
            d = per_core[c][r]
            ucol = n_gcols + r * CUNP
            n = np.arange(SP)
            idx[n % 128, ucol + n // 128] = d["rank"][n].astype(np.int32)
        coefnm = np.zeros((128, NUM_REL, NT), dtype=f32)
        for r in range(NUM_REL):
            d = per_core[c][r]
            cv = (1.0 / np.maximum(d["dcnt"][d["perm"]], 1)).astype(f32)  # sorted order
            coefnm[:, r, :] = cv.reshape(NT, 128).T
        in_maps.append({
            "desT": desT, "twT": twT, "nct": nct, "idx": idx, "coefnm": coefnm,
            "wd": wd, "wt": wt, "wnc": wnc, "wsm": wsm,
            "id128": id128, "id32": id32,
        })
    return cfg, in_maps, perm0s


_CACHE = {}
LAST_RESULT = None


def kernel(**inputs):
    global LAST_RESULT
    cfg, in_maps, perm0s = _prep(inputs)
    key = tuple((c0, ck) for r in range(NUM_REL) for (c0, ck, _) in cfg.pieces[r])
    if key not in _CACHE:
        _CACHE[key] = build_bass(cfg)
    nc = _CACHE[key]
    res = run_bass_kernel_spmd(nc, in_maps, list(range(N_CORES)))
    LAST_RESULT = res
    outs = []
    for c in range(N_CORES):
        o = res.results[c]["out"].T  # [SP, 2], rows in sigma0 order
        full = np.empty((cfg.shard_real, 2), dtype=np.float32)
        p0 = perm0s[c]
        sel = p0[p0 < cfg.shard_real]
        full[sel] = o[0:cfg.shard_real]
        outs.append(full)
    return np.ascontiguousarray(np.concatenate(outs, axis=0).astype(np.float32))

